# revision 1
# baseline (speedup 1.0000x reference)
"""BitNet attention (B=2, S=2048, HID=2560, NH=20, NKV=5, HD=128, GQA=4) on 8 TRN2 cores.

Sharding: 2-way batch x 4-way head-group tensor parallel.
Core (b, g) computes q-heads [4g, 4g+1, 4g+2, 4g+3, 16+g] and kv-heads [g, 4]
for batch b (uniform q-head -> kv mapping across cores so one SPMD NEFF works:
local heads 0-3 -> kv slot 0, local head 4 -> kv slot 1).

Per-core device pipeline, fused per 512-wide seq block j (causal => attention
for block j only needs K/V of blocks <= j):
  - Q^T/K^T = W@X^T (bf16 matmuls, exact ternary weights), RoPE on DVE
  - V = X@Wv^T in natural (seq, hd) layout
  - per head: S^T = K^T.T@Q^T (fp32r), exp on ACT (scale=alpha), causal binary
    mask on diag tiles, AV + softmax-denominator (ones-matmul) in PSUM,
    normalize via K=1 broadcast matmul + reciprocal, sum-of-squares for RMSNorm
    via ones-matmul; per-head tail chains are software-pipelined one head late.
  - o-proj partials y^T = Wo'@(w * attn_out^T) in bf16.
Host: unpack ternary weights, build RoPE tables, sum partial y / sumsq over the
4 cores of each batch, apply v/o scales and the RMSNorm per-seq scale (per-seq
scalars commute through the linear o-proj).
"""

import math
import numpy as np
import ml_dtypes
from contextlib import ExitStack

import concourse.bacc as bacc
import concourse.tile as tile
import concourse.mybir as mybir
from concourse import bass_utils

B, S, HID = 2, 2048, 2560
NH, NKV, HD = 20, 5, 128
THETA = 500000.0
RMS_EPS = 1e-6

N_CORES = 8
KT = HID // 128          # 20 k-tiles over hidden dim
J = S // 512             # 4 seq blocks of 512
SKT = S // 128           # 16 sk tiles
NQH = 5                  # q heads per core
NKVH = 2                 # kv heads per core

F32 = mybir.dt.float32
F32R = mybir.dt.float32r
BF16 = mybir.dt.bfloat16
F16 = mybir.dt.float16

_cache = {}

# schedule tuning knobs (sim-swept)
TUNE = {"depth": 4, "bc_at": 4, "b1_at": 5, "c_at": 6, "gp_mask": 0}


def _build(alpha: float, repeats: int):
    nc = bacc.Bacc("TRN2", target_bir_lowering=False, debug=False, num_devices=N_CORES)

    xt_d = nc.dram_tensor("xt", [HID, S], BF16, kind="ExternalInput")
    wq_d = nc.dram_tensor("wq", [HID, NQH * HD], BF16, kind="ExternalInput")
    wk_d = nc.dram_tensor("wk", [HID, NKVH * HD], BF16, kind="ExternalInput")
    wv_d = nc.dram_tensor("wv", [HID, NKVH * HD], BF16, kind="ExternalInput")
    wo_d = nc.dram_tensor("wo", [NQH * HD, HID], BF16, kind="ExternalInput")
    cos_d = nc.dram_tensor("cos", [HD, S], F16, kind="ExternalInput")
    sin_d = nc.dram_tensor("sin", [HD, S], F16, kind="ExternalInput")
    wn_d = nc.dram_tensor("wn", [HD, NQH], F32, kind="ExternalInput")
    dmask_d = nc.dram_tensor("dmask", [4, HD, 512], BF16, kind="ExternalInput")
    onc_d = nc.dram_tensor("onc", [HD, 1], F32R, kind="ExternalInput")
    onr_d = nc.dram_tensor("onr", [1, HD], F32R, kind="ExternalInput")
    y_d = nc.dram_tensor("y", [HID, S], F32, kind="ExternalOutput")
    ssq_d = nc.dram_tensor("ssq", [1, S], F32, kind="ExternalOutput")

    with tile.TileContext(nc) as tc, ExitStack() as octx:
        ps = octx.enter_context(tc.tile_pool(name="ps", bufs=8, space="PSUM"))
        kt_p = octx.enter_context(tc.tile_pool(name="ktp", bufs=1))
        v_p = octx.enter_context(tc.tile_pool(name="vp", bufs=1))
        qb_p = octx.enter_context(tc.tile_pool(name="qbp", bufs=6))
        const_p = octx.enter_context(tc.tile_pool(name="constp", bufs=1))
        w_p = octx.enter_context(tc.tile_pool(name="wp", bufs=1))
        xt_p = octx.enter_context(tc.tile_pool(name="xtp", bufs=2))
        rp_p = octx.enter_context(tc.tile_pool(name="rpp", bufs=6))
        pr_p = octx.enter_context(tc.tile_pool(name="prp", bufs=6))
        tw_p = octx.enter_context(tc.tile_pool(name="twp", bufs=7))
        mis_p = octx.enter_context(tc.tile_pool(name="misp", bufs=2))
        y_p = octx.enter_context(tc.tile_pool(name="yp", bufs=4))

        def body(_it=None):
            # --- persistent SBUF for one iteration ---
            kt = kt_p.tile([128, NKVH * S], BF16, tag="kt", name="kt")
            vt = v_p.tile([128, SKT * NKVH * HD], BF16, tag="vt", name="vt")

            wq = w_p.tile([128, KT * NQH * HD], BF16, tag="wq", name="wq")
            wk = w_p.tile([128, KT * NKVH * HD], BF16, tag="wk", name="wk")
            wv = w_p.tile([128, KT * NKVH * HD], BF16, tag="wv", name="wv")
            wo = w_p.tile([128, NQH * HID], BF16, tag="wo", name="wo")

            def dma_w_chunk(dst, src_d, W, k0, k1):
                nc.sync.dma_start(
                    dst[:, k0 * W:k1 * W].rearrange("p (k o) -> p k o", k=k1 - k0),
                    src_d.ap()[k0 * 128:k1 * 128].rearrange("(k p) o -> p k o", p=128))

            def dma_xt_chunk(dst, j, k0, k1):
                nc.sync.dma_start(
                    dst[:, k0 * 512:k1 * 512].rearrange("p (k s) -> p k s", k=k1 - k0),
                    xt_d.ap()[k0 * 128:k1 * 128, j * 512:(j + 1) * 512]
                    .rearrange("(k p) s -> p k s", p=128))

            xts = [None] * J

            # first compute chunk's data first, then the rest interleaved
            xts[0] = xt_p.tile([128, KT * 512], BF16, tag="xt", name="xt0")
            dma_xt_chunk(xts[0], 0, 0, 5)
            dma_w_chunk(wq, wq_d, NQH * HD, 0, 5)
            dma_w_chunk(wk, wk_d, NKVH * HD, 0, 5)
            dma_w_chunk(wv, wv_d, NKVH * HD, 0, 5)
            for c in range(1, 4):
                dma_xt_chunk(xts[0], 0, 5 * c, 5 * c + 5)
                dma_w_chunk(wq, wq_d, NQH * HD, 5 * c, 5 * c + 5)
                dma_w_chunk(wk, wk_d, NKVH * HD, 5 * c, 5 * c + 5)
                dma_w_chunk(wv, wv_d, NKVH * HD, 5 * c, 5 * c + 5)

            # constants / tables (needed slightly later than the first matmuls)
            cos_t = const_p.tile([HD, S], F16, tag="cos", name="cos")
            nc.sync.dma_start(cos_t[:], cos_d.ap())
            sin_t = const_p.tile([HD, S], F16, tag="sin", name="sin")
            nc.sync.dma_start(sin_t[:], sin_d.ap())
            onc = const_p.tile([HD, 1], F32R, tag="onc", name="onc")
            nc.sync.dma_start(onc[:], onc_d.ap())
            onr = const_p.tile([1, HD], F32R, tag="onr", name="onr")
            nc.sync.dma_start(onr[:], onr_d.ap())
            wn = const_p.tile([HD, NQH], F32, tag="wn", name="wn")
            nc.sync.dma_start(wn[:], wn_d.ap())
            dmask = const_p.tile([HD, 4 * 512], BF16, tag="dmask", name="dmask")
            for o in range(4):
                nc.sync.dma_start(dmask[:, o * 512:(o + 1) * 512], dmask_d.ap()[o])

            onc_bf = const_p.tile([HD, 1], BF16, tag="oncb", name="oncb")
            nc.any.memset(onc_bf[:], 1.0)

            pendA = [None]   # (h, j, d_ps, av_ps, ssq_ps, tws)
            pendBC = [None]  # (h, j, drow, av_ps, ssq_ps, tws)
            pendB = [None]   # (h, j, dbc, av_ps, ssq_ps, tws)

            def emit_tail_a():
                if pendA[0] is None:
                    return
                h, j, d_ps, av_ps, ssq_ps, tws = pendA[0]
                pendA[0] = None
                drow = mis_p.tile([1, 512], F32R, tag="drow", name=f"dr{j}_{h}")
                nc.scalar.copy(drow[:], d_ps[:])
                pendBC[0] = (h, j, drow, av_ps, ssq_ps, tws)

            def emit_tail_bc():
                if pendBC[0] is None:
                    return
                h, j, drow, av_ps, ssq_ps, tws = pendBC[0]
                pendBC[0] = None
                dbc = ps.tile([128, 512], F32, tag="ps", name=f"db{j}_{h}")
                nc.tensor.matmul(dbc[:], onr[:], drow[:], start=True, stop=True)
                pendB[0] = (h, j, dbc, av_ps, ssq_ps, tws)

            pendC = [None]  # (h, j, tn, ssq_ps, tws)

            def emit_tail_b():
                # B1: normalize -> frees av_ps psum slot; DVE only
                if pendB[0] is None:
                    return
                h, j, dbc, av_ps, ssq_ps, tws = pendB[0]
                pendB[0] = None
                rec = mis_p.tile([128, 512], F32, tag="rec", name=f"rc{j}_{h}")
                nc.vector.reciprocal(rec[:], dbc[:])
                tn = mis_p.tile([128, 512], F32, tag="tn", name=f"tn{j}_{h}")
                nc.vector.tensor_mul(tn[:], av_ps[:], rec[:])
                pendC[0] = (h, j, tn, ssq_ps, tws)

            def emit_tail_c():
                # B2: sumsq matmul + norm-weight scale
                if pendC[0] is None:
                    return
                h, j, tn, ssq_ps, tws = pendC[0]
                pendC[0] = None
                sqt = mis_p.tile([128, 512], F32R, tag="sqt", name=f"sq{j}_{h}")
                nc.scalar.square(sqt[:], tn[:])
                nc.tensor.matmul(ssq_ps[:], onc[:], sqt[:],
                                 start=(h == 0), stop=(h == NQH - 1))
                tw = tw_p.tile([128, 512], BF16, tag="tw", name=f"tw{j}_{h}")
                nc.vector.tensor_scalar_mul(tw[:], tn[:], wn[:, h:h + 1])
                tws.append(tw)

            for j in range(J):
                sq = slice(j * 512, (j + 1) * 512)
                xt = xts[j]
                if xt is None:
                    xt = xts[j] = xt_p.tile([128, KT * 512], BF16, tag="xt",
                                            name=f"xt{j}")
                    for c in range(4):
                        dma_xt_chunk(xt, j, 5 * c, 5 * c + 5)

                # ---- projections q/k for this block ----
                qbs = [qb_p.tile([128, 512], BF16, tag="qb", name=f"qb{j}_{h}")
                       for h in range(NQH)]
                ps_q = [ps.tile([128, 512], F32, tag="ps", name=f"pq{j}_{m}")
                        for m in range(NQH)]
                ps_k = [ps.tile([128, 512], F32, tag="ps", name=f"pk{j}_{m}")
                        for m in range(NKVH)]
                for k in range(KT):
                    xk = xt[:, k * 512:(k + 1) * 512]
                    st, sp = (k == 0), (k == KT - 1)
                    for m in range(NQH):
                        nc.tensor.matmul(
                            ps_q[m][:],
                            wq[:, k * 640 + m * 128: k * 640 + (m + 1) * 128],
                            xk, start=st, stop=sp)
                    for m in range(NKVH):
                        nc.tensor.matmul(
                            ps_k[m][:],
                            wk[:, k * 256 + m * 128: k * 256 + (m + 1) * 128],
                            xk, start=st, stop=sp)
                if j == 0:
                    # wo needed only at the first o-proj; start its DMA now
                    nc.sync.dma_start(
                        wo[:].rearrange("p (h o) -> p h o", h=NQH),
                        wo_d.ap().rearrange("(h p) o -> p h o", p=128))

                # ---- RoPE: ACT evac frees PSUM; trot halves read PSUM (DVE
                # cross-partition needs a PSUM input); the rest is same-partition
                # SBUF math that can be deferred just-in-time ----
                kdst = [kt[:, i * S + j * 512: i * S + (j + 1) * 512]
                        for i in range(NKVH)]
                for i in range(NKVH):
                    nc.scalar.copy(kdst[i], ps_k[i][:])
                for i in range(NQH):
                    nc.scalar.copy(qbs[i][:], ps_q[i][:])

                def rot_evac(psrc, idx):
                    # rotate-half via two ACT cross-partition copies (PSUM->SBUF)
                    qr = rp_p.tile([128, 512], F16, tag="trot", name=f"tr{j}_{idx}")
                    nc.scalar.copy(qr[0:64, :], psrc[64:128, :])
                    nc.scalar.copy(qr[64:128, :], psrc[0:64, :])
                    return qr

                def rope_math2(dst, qr):
                    nc.vector.tensor_mul(dst, dst, cos_t[:, sq])
                    nc.vector.tensor_mul(qr[:], qr[:], sin_t[:, sq])
                    nc.vector.tensor_add(dst, dst, qr[:])

                for i in range(NKVH):
                    qr = rot_evac(ps_k[i], i)
                    rope_math2(kdst[i], qr)
                trq = [rot_evac(ps_q[i], 2 + i) for i in range(NQH)]
                rope_math2(qbs[0][:], trq[0])

                # ---- V for this block ----
                ps_v = [ps.tile([128, NKVH * HD], F32, tag="ps", name=f"pv{j}_{t}")
                        for t in range(4)]
                for k in range(KT):
                    st, sp = (k == 0), (k == KT - 1)
                    for t in range(4):
                        nc.tensor.matmul(
                            ps_v[t][:],
                            xt[:, k * 512 + t * 128: k * 512 + (t + 1) * 128],
                            wv[:, k * 256:(k + 1) * 256],
                            start=st, stop=sp)
                for t in range(4):
                    i = 4 * j + t
                    nc.scalar.copy(vt[:, i * 256:(i + 1) * 256], ps_v[t][:])
                rope_math2(qbs[1][:], trq[1])
                rope_math2(qbs[2][:], trq[2])

                # ---- attention ----
                ni = 4 * j + 4
                ssq_ps = ps.tile([1, 512], F32, tag="ps", name=f"pss{j}")
                tws = []
                for h in range(NQH):
                    kvl = 0 if h < 4 else 1
                    qr = qbs[h][:]
                    av_ps = ps.tile([128, 512], F32, tag="ps", name=f"pav{j}_{h}")
                    d_ps = ps.tile([1, 512], F32, tag="ps", name=f"pd{j}_{h}")
                    queue = []

                    def flush_one():
                        pp, pi, pc0 = queue.pop(0)
                        nc.tensor.matmul(
                            av_ps[:],
                            vt[:, pi * 256 + kvl * 128: pi * 256 + kvl * 128 + 128],
                            pp[:], start=(pi == 0), stop=(pi == ni - 1))
                        nc.tensor.matmul(
                            d_ps[:], onc_bf[:], pp[:],
                            start=(pi == 0), stop=(pi == ni - 1))

                    bc_at = min(TUNE["bc_at"], ni - 2)
                    b1_at = min(TUNE["b1_at"], ni - 1)
                    for i in range(ni):
                        s_ps = ps.tile([128, 512], F32, tag="ps", name=f"pS{j}_{h}_{i}")
                        nc.tensor.matmul(
                            s_ps[:],
                            kt[:, kvl * S + i * 128: kvl * S + (i + 1) * 128],
                            qr, start=True, stop=True)
                        if len(queue) >= TUNE["depth"]:
                            flush_one()
                        probs = pr_p.tile([128, 512], BF16, tag="probs",
                                          name=f"pr{j}_{h}_{i}")
                        nc.scalar.activation(
                            probs[:], s_ps[:],
                            mybir.ActivationFunctionType.Exp, scale=alpha)
                        if i >= 4 * j:
                            o = i - 4 * j
                            nc.vector.tensor_mul(
                                probs[:], probs[:],
                                dmask[:, o * 512:(o + 1) * 512])
                        queue.append((probs, i, 0))
                        if i == 1:
                            emit_tail_a()   # drow copy (ACT)
                        if i == bc_at:
                            emit_tail_bc()  # broadcast matmul (PE)
                        if i == b1_at:
                            emit_tail_b()   # recip+normalize: frees av slot
                        if i == TUNE["c_at"]:
                            emit_tail_c()   # sumsq + tw
                    while queue:
                        flush_one()
                    emit_tail_c()  # short blocks: flush after the AV/d drain
                    if h + 3 < NQH:
                        rope_math2(qbs[h + 3][:], trq[h + 3])  # just-in-time
                    pendA[0] = (h, j, d_ps, av_ps, ssq_ps, tws)

                srow = mis_p.tile([1, 512], F32, tag="srow", name=f"sr{j}")
                emit_tail_a()  # last head's drow copy (ACT)

                # ---- o-proj; last head's tail overlapped inside first chunk ----
                chunks = [(0, 4), (4, 6), (6, 8), (8, 10), (10, 12), (12, 14),
                          (14, 16), (16, 18), (18, 20)]
                first = True
                for (m0, m1) in chunks:
                    y_pss = [ps.tile([128, 512], F32, tag="ps", name=f"py{j}_{m}")
                             for m in range(m0, m1)]
                    for h in range(NQH):
                        if first and h == 1:
                            emit_tail_bc()
                        if first and h == 2:
                            emit_tail_b()
                        if first and h == 3:
                            emit_tail_c()
                            nc.scalar.copy(srow[:], ssq_ps[:])
                            nc.sync.dma_start(ssq_d.ap()[:, sq], srow[:])
                            first = False
                        for mi, m in enumerate(range(m0, m1)):
                            nc.tensor.matmul(
                                y_pss[mi][:],
                                wo[:, h * HID + m * 128: h * HID + (m + 1) * 128],
                                tws[h][:], start=(h == 0), stop=(h == NQH - 1))
                    for mi, m in enumerate(range(m0, m1)):
                        ysb = y_p.tile([128, 512], F32, tag="ysb", name=f"y{j}_{m}")
                        if m % 2 == 0:
                            nc.scalar.copy(ysb[:], y_pss[mi][:])
                        else:
                            nc.vector.tensor_copy(ysb[:], y_pss[mi][:])
                        nc.sync.dma_start(
                            y_d.ap()[m * 128:(m + 1) * 128, sq], ysb[:])

                # prefetch next block's activations
                if j + 1 < J:
                    xts[j + 1] = xt_p.tile([128, KT * 512], BF16, tag="xt",
                                           name=f"xt{j+1}")
                    for c in range(4):
                        dma_xt_chunk(xts[j + 1], j + 1, 5 * c, 5 * c + 5)

        if repeats > 1:
            with tc.For_i(0, repeats) as _i:
                body(_i)
        else:
            body()

    nc.compile()
    return nc


def _unpack_ternary(packed: np.ndarray) -> np.ndarray:
    M, Kp = packed.shape
    nb = Kp // 32
    b = packed.reshape(M, nb, 32)
    f = np.stack([(b >> 6) & 3, (b >> 4) & 3, (b >> 2) & 3, b & 3], axis=2)
    return f.reshape(M, nb * 128).astype(np.float32) - 1.0


def _rope_tables():
    inv = 1.0 / (THETA ** (np.arange(0, HD, 2, dtype=np.float64) / HD))  # (64,)
    t = np.arange(S, dtype=np.float64)
    fr = t[None, :] * inv[:, None]          # (64, S)
    cos = np.concatenate([np.cos(fr), np.cos(fr)], axis=0)      # (128, S)
    sin = np.concatenate([-np.sin(fr), np.sin(fr)], axis=0)     # signed
    return cos.astype(np.float16), sin.astype(np.float16)


def _diag_masks():
    m = np.zeros((4, HD, 512), dtype=ml_dtypes.bfloat16)
    q = np.arange(512)[None, :]
    p = np.arange(HD)[:, None]
    for o in range(4):
        m[o] = (q >= p + 128 * o).astype(ml_dtypes.bfloat16)
    return m


def make_in_maps(hidden_states, q_w, k_w, v_w, o_w, attn_norm_w):
    wq_f = _unpack_ternary(np.asarray(q_w))     # (2560, 2560)
    wk_f = _unpack_ternary(np.asarray(k_w))     # (640, 2560)
    wv_f = _unpack_ternary(np.asarray(v_w))     # (640, 2560)
    wo_f = _unpack_ternary(np.asarray(o_w))     # (2560, 2560) [out, in]
    cos, sin = _rope_tables()
    dmask = _diag_masks()
    onc = np.ones((HD, 1), np.float32)
    onr = np.ones((1, HD), np.float32)
    wnorm = np.asarray(attn_norm_w, np.float32)
    hs = np.asarray(hidden_states)

    bf = ml_dtypes.bfloat16
    in_maps = []
    for c in range(N_CORES):
        b, g = c // 4, c % 4
        qheads = [4 * g, 4 * g + 1, 4 * g + 2, 4 * g + 3, 16 + g]
        kvheads = [g, 4]
        qrows = np.concatenate([wq_f[h * HD:(h + 1) * HD] for h in qheads], 0)
        krows = np.concatenate([wk_f[h * HD:(h + 1) * HD] for h in kvheads], 0)
        vrows = np.concatenate([wv_f[h * HD:(h + 1) * HD] for h in kvheads], 0)
        ocols = np.concatenate([wo_f[:, h * HD:(h + 1) * HD] for h in qheads], 1)
        wn = np.stack([wnorm[h * HD:(h + 1) * HD] for h in qheads], 1)  # (128, 5)
        in_maps.append({
            "xt": np.ascontiguousarray(hs[b].T).astype(bf),
            "wq": np.ascontiguousarray(qrows.T).astype(bf),
            "wk": np.ascontiguousarray(krows.T).astype(bf),
            "wv": np.ascontiguousarray(vrows.T).astype(bf),
            "wo": np.ascontiguousarray(ocols.T).astype(bf),
            "cos": cos, "sin": sin,
            "wn": np.ascontiguousarray(wn),
            "dmask": dmask, "onc": onc, "onr": onr,
        })
    return in_maps


def postprocess(results, v_scale, o_scale):
    out = np.empty((B, S, HID), np.float32)
    for b in range(B):
        y = np.zeros((HID, S), np.float64)
        ss = np.zeros((S,), np.float64)
        for g in range(4):
            r = results[b * 4 + g]
            y += r["y"].astype(np.float64)
            ss += r["ssq"][0].astype(np.float64)
        var = ss * (float(v_scale) ** 2) / HID
        rms = 1.0 / np.sqrt(var + RMS_EPS)
        out[b] = (y.T * (rms[:, None] * float(v_scale) * float(o_scale))).astype(np.float32)
    return out


def _get_nc(alpha: float, repeats: int = 1):
    key = (round(alpha, 12), repeats)
    if key not in _cache:
        _cache[key] = _build(alpha, repeats)
    return _cache[key]


def kernel(hidden_states, attention_mask, q_w, k_w, v_w, o_w,
           q_scale, k_scale, v_scale, o_scale, attn_norm_w):
    alpha = float(q_scale) * float(k_scale) / math.sqrt(HD)
    nc = _get_nc(alpha, 1)
    in_maps = make_in_maps(hidden_states, q_w, k_w, v_w, o_w, attn_norm_w)
    res = bass_utils.run_bass_kernel_spmd(nc, in_maps, core_ids=list(range(N_CORES)))
    return postprocess(res.results, v_scale, o_scale)

